# revision 11
# baseline (speedup 1.0000x reference)
"""Trainium2 Bass kernel for nn_BaseBox2dHead (sigmoid -> topk -> NMS detection head).

Contract: kernel(**inputs) takes the FULL inputs (cls_logits [2,200000,81] f32,
pred_boxes [2,200000,4] f32) and returns the reference's full output tuple
(labels [200] i32, boxes [200,4] f32, scores [200] f32, batch_ids [200] i32).

Sharding: 8 shards = 2 images x 4 row-chunks of 50000 boxes; one shard per
NeuronCore (data-parallel, no cross-device communication).

Device kernel (the memory-roofline work): each core streams its full shard of
logits (+boxes) from HBM and reduces each box-row of 80 class logits to a
single masked row-max (background class and padding pre-masked to -1e30,
ill-defined boxes masked on device). 17MB in -> 200KB out per core.

Host glue (O(K) only): the top-1000 candidate (row,class) pairs of an image
provably live in the top-1000 rows ranked by masked row-max, because a
candidate's score is <= its own row-max, so any row holding a top-1000
candidate has row-max >= the 1000th best candidate >= the 1000th best row-max.
The host therefore selects the top-1200 rows per image, gathers just those
rows' logits/boxes, rebuilds exact scores (sigmoid on CPU, matching the
reference), takes the exact top-1000 with the reference's tie rule, runs the
tiny 1000-box greedy NMS, and formats the output.
"""

import os

import numpy as np

import concourse.bass as bass
import concourse.mybir as mybir
import concourse.tile as tile
from concourse import bass_utils

# ---------------------------------------------------------------- constants
B = 2
N = 200000
L = 81
C = 80
NMS_CANDIDATES = 1000
NMS_THR = 0.65
MAX_DETS = 100

N_CORES = 8
SHARD = N // 4          # 50000 rows per core
P = 128                 # SBUF partitions
R = 98                  # rows per partition per tile
T_TILES = 4
ROWS_PER_TILE = P * R   # 12544
PAD_ROWS = T_TILES * ROWS_PER_TILE  # 50176

NEG = np.float32(-1.0e30)

# Extra rows fetched beyond the provably-sufficient 1000, to make fp ties at
# the boundary a non-issue.
ROW_MARGIN = 1200

LAST_RESULTS = None  # BassKernelResults of the most recent device run
_CACHE = {}


# ---------------------------------------------------------------- device IR
def _build_nc():
    """Per-core program: stream [PAD_ROWS, 81] logits + [PAD_ROWS, 4] boxes,
    emit masked per-row max [128, T_TILES, R] (rowmax[p,t,j] = row
    t*ROWS_PER_TILE + p*R + j).

    Raw Bass (no TileContext): every SBUF buffer is written exactly once, so
    the only syncs needed are DMA-completion waits on the vector engine and
    one vector-completion wait before the output DMA. Explicit wait_ge
    instructions also sidestep the TRN2 single-embedded-wait DMA encoding
    limit that Tile's auto-sync runs into here.
    """
    from contextlib import ExitStack

    # detect_race_conditions=False: the sim's race detector doesn't credit
    # same-engine program order for raw (non-Tile) streams, but TRN2 DVE
    # flushes its pipe after every op, so in-order same-engine RAW is safe.
    nc = bass.Bass(
        "TRN2", target_bir_lowering=False, debug=False,
        detect_race_conditions=False,
    )
    f32 = mybir.dt.float32
    lg = nc.dram_tensor("logits", [PAD_ROWS, L], f32, kind="ExternalInput").ap()
    bx = nc.dram_tensor("boxes", [PAD_ROWS, 4], f32, kind="ExternalInput").ap()
    out = nc.dram_tensor("rowmax", [P, T_TILES, R], f32, kind="ExternalOutput").ap()

    with ExitStack() as ctx:
        lts = [
            ctx.enter_context(nc.sbuf_tensor(f"lt{t}", [P, R * L], f32))
            for t in range(T_TILES)
        ]
        ball = ctx.enter_context(nc.sbuf_tensor("ball", [P, T_TILES * R * 4], f32))
        rm_all = ctx.enter_context(nc.sbuf_tensor("rm_all", [P, T_TILES * R], f32))
        wx = ctx.enter_context(nc.sbuf_tensor("wx", [P, R], f32))
        wy = ctx.enter_context(nc.sbuf_tensor("wy", [P, R], f32))
        dma_sem = ctx.enter_context(nc.semaphore())
        vec_sem = ctx.enter_context(nc.semaphore())
        blk = ctx.enter_context(nc.Block())

        @blk.sync
        def _(sync):
            # all boxes in one DMA, laid out to match the logits tiling:
            # partition p holds (t, r, c)
            sync.dma_start(
                out=ball.ap().rearrange("p (t x) -> p t x", t=T_TILES),
                in_=bx.rearrange("(t p r) c -> p t (r c)", t=T_TILES, p=P),
            ).then_inc(dma_sem, 16)
            for t in range(T_TILES):
                rows = slice(t * ROWS_PER_TILE, (t + 1) * ROWS_PER_TILE)
                sync.dma_start(
                    out=lts[t].ap(),
                    in_=lg[rows, :].rearrange("(p r) c -> p (r c)", p=P),
                ).then_inc(dma_sem, 16)
            sync.wait_ge(vec_sem, T_TILES)
            sync.dma_start(
                out=out.rearrange("p t r -> p (t r)"), in_=rm_all.ap()
            ).then_inc(dma_sem, 16)
            sync.wait_ge(dma_sem, 16 * (T_TILES + 2))

        @blk.vector
        def _(vector):
            b4 = ball.ap().rearrange("p (t r c) -> p t r c", t=T_TILES, c=4)
            for t in range(T_TILES):
                rm = rm_all.ap()[:, t * R : (t + 1) * R]
                # boxes (+16) and logits tiles 0..t (+16 each) have landed
                vector.wait_ge(dma_sem, 16 * (t + 2))
                vector.tensor_reduce(
                    rm,
                    lts[t].ap().rearrange("p (r c) -> p r c", c=L),
                    axis=mybir.AxisListType.X,
                    op=mybir.AluOpType.max,
                )
                # well-defined mask: (x2 > x1) & (y2 > y1); fold into the
                # row max as rm = min(rm, wd * 2e30 - 1e30)
                vector.tensor_tensor(
                    wx.ap(), b4[:, t, :, 2], b4[:, t, :, 0],
                    op=mybir.AluOpType.is_gt,
                )
                vector.tensor_tensor(
                    wy.ap(), b4[:, t, :, 3], b4[:, t, :, 1],
                    op=mybir.AluOpType.is_gt,
                )
                vector.tensor_tensor(
                    wx.ap(), wx.ap(), wy.ap(), op=mybir.AluOpType.mult
                )
                vector.tensor_scalar(
                    wx.ap(), wx.ap(), 2.0e30, -1.0e30,
                    mybir.AluOpType.mult, mybir.AluOpType.add,
                )
                vector.tensor_tensor(
                    rm, rm, wx.ap(), op=mybir.AluOpType.min
                ).then_inc(vec_sem, 1)
    return nc


def _get_nc():
    if "nc" not in _CACHE:
        _CACHE["nc"] = _build_nc()
    return _CACHE["nc"]


# ---------------------------------------------------------------- host glue
def _sigmoid32(x):
    """f32 sigmoid, bit-identical to the jax-on-CPU reference when possible."""
    try:
        import jax

        cpu = jax.devices("cpu")[0]
        with jax.default_device(cpu):
            return np.asarray(jax.jit(jax.nn.sigmoid)(x))
    except Exception:
        x = np.asarray(x, np.float32)
        return (np.float32(1.0) / (np.float32(1.0) + np.exp(-x))).astype(np.float32)


def _divmod_like_reference(top_idx):
    """Replicate the reference's `top_idx // C` and `top_idx % C`.

    XLA on CPU lowers int32 divide/remainder by 80 through a float fast path:
    for indices with remainder 79 (e.g. 15786239) it yields quotient
    (idx+1)//80 and remainder -1, so the reference gathers the next row's box
    and emits label -1. Computing with the same jax ops reproduces this
    bit-for-bit. Falls back to a direct emulation of the observed rounding if
    jax is unavailable.
    """
    top_idx = np.asarray(top_idx, np.int32)
    try:
        import jax
        import jax.numpy as jnp

        cpu = jax.devices("cpu")[0]
        with jax.default_device(cpu):
            ti = jnp.asarray(top_idx)
            rows = np.asarray(jax.jit(lambda t: t // C)(ti))
            labels = np.asarray(jax.jit(lambda t: t % C)(ti))
    except Exception:
        q = np.float32(top_idx) / np.float32(C)
        rows = np.round(q).astype(np.int32)
        labels = (top_idx - rows * C).astype(np.int32)
        neg = labels < -1  # only the round-to-nearest quotient is emulated
        rows = np.where(neg, top_idx // C, rows)
        labels = np.where(neg, top_idx % C, labels)
    return rows.astype(np.int64), labels.astype(np.int32)


def _greedy_nms(boxes, labels):
    """Exact replication of the reference greedy class-aware NMS (f32)."""
    K = boxes.shape[0]
    max_coord = boxes.max()
    off = labels.astype(np.float32)[:, None] * (max_coord + np.float32(1.0))
    bb = boxes + off
    x1, y1, x2, y2 = bb[:, 0], bb[:, 1], bb[:, 2], bb[:, 3]
    area = (x2 - x1) * (y2 - y1)
    ix1 = np.maximum(x1[:, None], x1[None, :])
    iy1 = np.maximum(y1[:, None], y1[None, :])
    ix2 = np.minimum(x2[:, None], x2[None, :])
    iy2 = np.minimum(y2[:, None], y2[None, :])
    inter = np.clip(ix2 - ix1, 0.0, None) * np.clip(iy2 - iy1, 0.0, None)
    iou = inter / (area[:, None] + area[None, :] - inter + np.float32(1e-12))
    sup = iou > np.float32(NMS_THR)
    np.fill_diagonal(sup, False)
    keep = np.ones(K, dtype=bool)
    idx = np.arange(K)
    # only rows that suppress anything matter; iterate them in score order
    for i in np.nonzero(sup.any(axis=1))[0]:
        if keep[i]:
            keep &= ~(sup[i] & (idx > i))
    return keep


def _detect_one(rowmax_img, logits_img, boxes_img):
    """rowmax_img: [N] masked row maxima; logits/boxes: full image arrays."""
    nrows = min(ROW_MARGIN, rowmax_img.size)
    rows = np.argpartition(-rowmax_img, nrows - 1)[:nrows]
    lgr = logits_img[rows][:, :C]
    bxr = boxes_img[rows]
    wd = (bxr[:, 2] > bxr[:, 0]) & (bxr[:, 3] > bxr[:, 1])
    sc = _sigmoid32(lgr) * wd[:, None].astype(np.float32)
    flat = (rows.astype(np.int64)[:, None] * C + np.arange(C)[None, :]).ravel()
    scf = sc.ravel()
    # top-1000 with the reference tie rule (score desc, flat index asc)
    order = np.lexsort((flat, -scf))[:NMS_CANDIDATES]
    top_scores = scf[order]
    top_idx = flat[order].astype(np.int32)
    cand_rows, cand_labels = _divmod_like_reference(top_idx)
    # jnp gather clamps out-of-range indices (the quirk can round row N-1 up
    # to N at the array edge)
    cand_boxes = boxes_img[np.clip(cand_rows, 0, boxes_img.shape[0] - 1)]
    keep = _greedy_nms(cand_boxes, cand_labels)
    sel = np.where(keep, top_scores, np.float32(-1.0))
    det_idx = np.lexsort((np.arange(sel.size), -sel))[:MAX_DETS]
    det_keep = keep[det_idx]
    det_scores = np.where(det_keep, top_scores[det_idx], np.float32(0.0))
    det_labels = np.where(det_keep, cand_labels[det_idx], np.int32(-1)).astype(
        np.int32
    )
    det_boxes = cand_boxes[det_idx] * det_keep[:, None].astype(np.float32)
    return det_labels, det_boxes, det_scores


# ---------------------------------------------------------------- entry
def kernel(cls_logits, pred_boxes):
    global LAST_RESULTS
    cls_logits = np.ascontiguousarray(np.asarray(cls_logits, dtype=np.float32))
    pred_boxes = np.ascontiguousarray(np.asarray(pred_boxes, dtype=np.float32))
    assert cls_logits.shape == (B, N, L) and pred_boxes.shape == (B, N, 4)

    in_maps = []
    for core in range(N_CORES):
        b, q = divmod(core, 4)
        lg = np.full((PAD_ROWS, L), NEG, np.float32)
        lg[:SHARD] = cls_logits[b, q * SHARD : (q + 1) * SHARD]
        lg[:, C] = NEG  # background class never participates
        bx = np.zeros((PAD_ROWS, 4), np.float32)
        bx[:SHARD] = pred_boxes[b, q * SHARD : (q + 1) * SHARD]
        in_maps.append({"logits": lg, "boxes": bx})

    nc = _get_nc()
    trace = bool(int(os.environ.get("KERNEL_TRACE", "0")))
    res = bass_utils.run_bass_kernel_spmd(
        nc, in_maps, list(range(N_CORES)), trace=trace
    )
    LAST_RESULTS = res

    labels_all, boxes_all, scores_all = [], [], []
    for b in range(B):
        rm = np.concatenate(
            [
                np.transpose(
                    np.asarray(res.results[b * 4 + q]["rowmax"], np.float32),
                    (1, 0, 2),
                ).reshape(-1)[:SHARD]
                for q in range(4)
            ]
        )
        det_labels, det_boxes, det_scores = _detect_one(
            rm, cls_logits[b], pred_boxes[b]
        )
        labels_all.append(det_labels)
        boxes_all.append(det_boxes)
        scores_all.append(det_scores)

    labels = np.concatenate(labels_all).astype(np.int32)
    boxes = np.concatenate(boxes_all).astype(np.float32)
    scores = np.concatenate(scores_all).astype(np.float32)
    batch_ids = np.repeat(np.arange(B, dtype=np.int32), MAX_DETS)
    return labels, boxes, scores, batch_ids
